# revision 11
# baseline (speedup 1.0000x reference)
"""Causal self-attention (QK-RMSNorm + rotary + value-embed blend) on 8 trn2 cores.

Sharding: 8 cores = 4 batches x 2 head-halves (8 heads each).
Host pre-transposes/casts inputs to fp16; device computes a per-core output
partial [1024, 1024] fp32 (output projection contracted over its 512 hdim
columns); host sums the two partials per batch.

Per-core kernel structure, software-pipelined per head-pair g in 0..3:
  proj(g): q|k|v = x @ W[:, pair-cols]  (fp16 matmuls, one fp32 psum group)
  per token-half (4 t-tiles): RMS-norm scale (ACT square -> DVE reduce ->
  ln/exp rsqrt) applied on the fp16 copy, fused q+k rotary (one DVE op set
  over all 8 (tensor,head,half) groups), DRAM write + xbar transpose DMA
  chunk into QT/KT - all overlapping the remaining proj matmuls
  scores^T[kpos, q] = K^T q per (head, kt, 512-q-chunk) into a 2-bank psum
  (bufs=2 so exp(chunk) overlaps scores(chunk+1)); the diagonal 128-block
  gets an additive -C*max(0,k-q) ramp from a rank-128 mask matmul so exp
  kills the upper triangle with no elementwise mask op
  exp via ACT straight from psum (scale=0.12 fused), fp16, packed causally
  AV for pair g-1 (one-stage software pipeline so pair g's rms/rotary/
  transpose chain never waits behind pair g-1's AV divides on DVE, and the
  qk DMAs beat the y DMAs onto the SP queue): y[q,:] accumulates
  ET[kt]^T @ [V|1] over kt; softmax denominator from the appended ones
  column; divide via per-partition reciprocal + broadcast multiply;
  y written + transposed per token-half
Then out_partial = y @ woT per t-tile, stores alternating Pool/SP queues.

The PE clock p-state ramp (0.65->1.2->2.4GHz over 3us of continuous
execution) is pre-warmed with dummy matmuls during the input-DMA window.

Numerics: QK RMS-norm bounds scores (|s| <= 0.12*64), so softmax needs no
max-subtraction; rsqrt is computed as exp(-0.5*ln(x)) so ACT only ever loads
the ln/exp table set (no table thrash against the attention exp).
"""

import sys

if "/opt/trn_rl_repo" not in sys.path:
    sys.path.insert(0, "/opt/trn_rl_repo")

import numpy as np

import concourse.bass as bass
import concourse.mybir as mybir
import concourse.tile as tile
from concourse.bass import ds, ts

P = 128
B, T, D = 4, 1024, 1024
H, DH = 16, 64
H8 = 8            # heads per core
NG = H8 // 2      # head pairs
ATTN_SCALE = 0.12
EPS = 1e-6
N_CORES = 8
TT_N = T // P     # 8 t-tiles
DT_N = D // P     # 8 d-tiles (contraction)
KT_N = T // P     # 8 kpos tiles
RT_N = (H8 * DH) // P  # 4 row-tiles of Q^T/K^T
JT_N = (H8 * DH) // P  # 4 j-tiles for out-proj contraction

f16 = mybir.dt.float16
f32 = mybir.dt.float32


def split_sync_waits(nc, max_waits=1):
    """This container's walrus rejects >1 sync-wait per instruction; spill
    extra waits onto preceding NoOps on the same engine."""
    n = 0
    for fn in nc.m.functions:
        for bb in fn.blocks:
            new_insts = []
            for inst in bb.instructions:
                si = getattr(inst, "sync_info", None)
                if si is not None and si.on_wait and len(si.on_wait) > max_waits:
                    waits = list(si.on_wait)
                    extra, keep = waits[:-max_waits], waits[-max_waits:]
                    for w in extra:
                        nop = mybir.InstNoOp(
                            name=nc.get_next_instruction_name(),
                            sync_info=mybir.SyncInfo(on_wait=[w], on_update=[]),
                            bass_nofuse=True,
                            engine=inst.engine,
                        )
                        nc.register_instruction(nop)
                        new_insts.append(nop)
                        n += 1
                    si.on_wait = keep
                new_insts.append(inst)
            bb.instructions[:] = new_insts
    return n


def build_nc(reps=1):
    nc = bass.Bass()

    # xTt: block-transposed x so each token-tile chunk is one contiguous
    # descriptor per partition: xTt[tt*128+p, dt*128+j] = x[tt*128+j, dt*128+p]
    xTt = nc.declare_dram_parameter("xTt", [T, D], f16, isOutput=False)
    # per-pair contiguous fused qkv weights: [D, pair, (q|k|v)*128]
    wqkv = nc.declare_dram_parameter("wqkv", [D, NG, 384], f16, isOutput=False)
    woT = nc.declare_dram_parameter("woT", [H8 * DH, D], f16, isOutput=False)
    ve = nc.declare_dram_parameter("ve", [T, H8 * DH], f16, isOutput=False)
    # rotary tables pre-tiled to [P, TT_N*128] (partition-contiguous)
    cosd = nc.declare_dram_parameter("cosd", [P, TT_N * P], f16, isOutput=False)
    sind = nc.declare_dram_parameter("sind", [P, TT_N * P], f16, isOutput=False)
    # causal ramp-mask factors: (Am.T @ Bm)[k, q] = -C * max(0, k - q)
    amask = nc.declare_dram_parameter("amask", [P, P], f16, isOutput=False)
    bmask = nc.declare_dram_parameter("bmask", [P, P], f16, isOutput=False)
    out = nc.declare_dram_parameter("out", [T, D], f32, isOutput=True)

    xT_v = xTt.rearrange("(tt p) c -> p tt c", p=P)
    wqkv_v = wqkv.rearrange("(dt p) g r -> p dt g r", p=P)
    wo_v = woT.rearrange("(jt p) i -> p jt i", p=P)
    ve_v = ve.rearrange("(tt p) r -> p tt r", p=P)
    cos_v = cosd.rearrange("p (tt e) -> p tt e", e=P)
    sin_v = sind.rearrange("p (tt e) -> p tt e", e=P)
    out_v = out.rearrange("(tt p) i -> p tt i", p=P)

    with tile.TileContext(nc) as tc:
        import contextlib

        with contextlib.ExitStack() as ctx:
            const = ctx.enter_context(tc.tile_pool(name="const", bufs=1))
            big = ctx.enter_context(tc.tile_pool(name="big", bufs=1))

            # -------- persistent tiles --------
            xT_sb = big.tile([P, TT_N, DT_N, P], f16)  # [p, tt, dt, tok]
            wqkv_sb = const.tile([P, DT_N, NG, 384], f16)
            wo_sb = const.tile([P, JT_N, D], f16)
            ve_sb = const.tile([P, TT_N, H8 * DH], f16)
            cos_sb = const.tile([P, TT_N, P], f16)
            sin_sb = const.tile([P, TT_N, P], f16)
            am_sb = const.tile([P, P], f16)
            bm_sb = const.tile([P, P], f16)

            # loads split across 3 DMA queues so pair-0 proj starts ~4us:
            # SP: first wqkv half, xT tiles 1-4, rotary tables
            # ACT: xT tile 0 + masks (ACT engine idle at t=0)
            # Pool (swdge): second wqkv half, ve, xT 5-7, pairs 1-3, wo
            nc.sync.dma_start(
                out=wqkv_sb[:, 0:4, 0, :], in_=wqkv_v[:, 0:4, 0, :])
            nc.scalar.dma_start(out=xT_sb[:, 0], in_=xT_v[:, 0])
            nc.gpsimd.dma_start(
                out=wqkv_sb[:, 4:8, 0, :], in_=wqkv_v[:, 4:8, 0, :])
            for tt in range(1, 5):
                nc.sync.dma_start(out=xT_sb[:, tt], in_=xT_v[:, tt])
            nc.scalar.dma_start(out=am_sb[:], in_=amask[:])
            nc.scalar.dma_start(out=bm_sb[:], in_=bmask[:])
            nc.sync.dma_start(out=cos_sb[:], in_=cos_v[:])
            nc.sync.dma_start(out=sin_sb[:], in_=sin_v[:])
            nc.gpsimd.dma_start(out=ve_sb[:], in_=ve_v[:])
            for tt in range(5, TT_N):
                nc.gpsimd.dma_start(out=xT_sb[:, tt], in_=xT_v[:, tt])
            for g in range(1, NG):
                nc.gpsimd.dma_start(
                    out=wqkv_sb[:, :, g, :], in_=wqkv_v[:, :, g, :])
            nc.gpsimd.dma_start(out=wo_sb[:], in_=wo_v[:])

            vp = big.tile([P, KT_N, H8, DH + 1], f16)  # V blended + ones col
            y16 = big.tile([P, NG, TT_N, P], f16)
            QT_sb = big.tile([P, RT_N, T], f16)      # [r, t] fp16
            KT_sb = big.tile([P, RT_N, T], f16)
            yT_sb = big.tile([P, JT_N, T], f16)

            warm_sb = const.tile([P, P], f16)
            nc.vector.memset(warm_sb[:], 0.0)
            nc.vector.memset(vp[:, :, :, DH], 1.0)
            eps_sb = const.tile([P, 1], f32)
            nc.vector.memset(eps_sb[:], EPS)

            # PE warm-up during the input-DMA window: the cost model ramps the
            # PE clock 0.65->1.2->2.4GHz over 3us of *continuous* execution.
            with tc.tile_pool(name="warmps", bufs=1, space="PSUM") as wps:
                wp = wps.tile([P, P], f32, tag="warm")
                NWARM = 38
                for i in range(NWARM):
                    # one accumulation group: no psum drains between matmuls,
                    # so the PE stream is gapless and actually ramps
                    nc.tensor.matmul(
                        wp[:], warm_sb[:], warm_sb[:],
                        start=(i == 0), stop=(i == NWARM - 1))

            # ET packed causally: kt block holds q in [kt*128, T)
            etoff = [kt * T - 64 * kt * (kt - 1) for kt in range(KT_N + 1)]

            for _rep in range(reps):
                with tc.tile_pool(name="projps", bufs=2, space="PSUM") as psB, \
                     tc.tile_pool(name="stps", bufs=2, space="PSUM") as psST, \
                     tc.tile_pool(name="avps", bufs=2, space="PSUM") as psAV, \
                     tc.tile_pool(name="qk", bufs=2) as qkp, \
                     tc.tile_pool(name="sq", bufs=2) as sqp, \
                     tc.tile_pool(name="etp", bufs=2) as etp, \
                     tc.tile_pool(name="qkdr", bufs=2, space="DRAM") as qkdr, \
                     tc.tile_pool(name="stat", bufs=4) as statp:
                    et_tiles = {}

                    def emit_av(g):
                        """AV + divide + per-half y write/transpose, pair g."""
                        ET = et_tiles.pop(g)
                        y_dr = qkdr.tile([T, P], f16, tag="ydr")
                        y_dr_v = y_dr.rearrange("(tt p) r -> p tt r", p=P)
                        for q2 in range(TT_N // 2):
                            pav = psAV.tile([P, 260], f32, tag="av")
                            mms = []
                            for sub in range(2):
                                qt = 2 * q2 + sub
                                for hb in range(2):
                                    for kt in range(qt + 1):
                                        mms.append((sub, qt, hb, kt))
                            for i, (sub, qt, hb, kt) in enumerate(mms):
                                nc.tensor.matmul(
                                    pav[:, ds(sub * 130 + hb * 65, 65)],
                                    ET[:, hb, ds(etoff[kt] + (qt - kt) * P, P)],
                                    vp[:, kt, 2 * g + hb, :],
                                    start=(i == 0),
                                    stop=(i == len(mms) - 1),
                                )
                            pavv = pav.rearrange("p (s h c) -> p s h c", s=2, h=2)
                            r = statp.tile([P, 2, 2], f32, tag="r")
                            nc.vector.reciprocal(
                                r[:], pavv[:, :, :, DH : DH + 1])
                            nc.vector.tensor_tensor(
                                y16[:, g, ds(2 * q2, 2), :].rearrange(
                                    "p s (h e) -> p s h e", h=2),
                                pavv[:, :, :, 0:DH],
                                r[:, :, :, None].to_broadcast((P, 2, 2, DH)),
                                mybir.AluOpType.mult,
                            )
                            if g == NG - 1:
                                # last pair feeds the out-proj: finest chunks
                                th = ds(2 * q2, 2)
                                nc.sync.dma_start(
                                    out=y_dr_v[:, th, :], in_=y16[:, g, th, :])
                                nc.sync.dma_start_transpose(
                                    yT_sb[:, g, ds(256 * q2, 256)],
                                    y_dr[ds(256 * q2, 256), :])
                            elif q2 % 2 == 1:
                                hf = q2 // 2
                                th = ds(4 * hf, 4)
                                nc.sync.dma_start(
                                    out=y_dr_v[:, th, :], in_=y16[:, g, th, :])
                                nc.sync.dma_start_transpose(
                                    yT_sb[:, g, ds(512 * hf, 512)],
                                    y_dr[ds(512 * hf, 512), :])

                    for g in range(NG):
                        gc = ts(g, P)  # this pair's 128 cols in q/k/v rows
                        QKg = qkp.tile([P, TT_N, 2 * P], f16, tag="qkg")
                        sqg = sqp.tile([P, TT_N, 256], f16, tag="sqg")
                        qk_dr = qkdr.tile([T, 2 * P], f16, tag="qkdr")
                        qk_dr_v = qk_dr.rearrange("(tt p) r -> p tt r", p=P)
                        # ---------- projection (+v blend, raw qk copy) -------
                        for tt in range(TT_N):
                            pj = psB.tile([P, 384], f32, tag="pqkv")
                            # single psum accumulation group for the whole
                            # bank (q,k,v ranges interleave; per-element
                            # has_written handles first-write-overwrite)
                            for dt in range(DT_N):
                                lx = xT_sb[:, tt, dt, :]
                                nc.tensor.matmul(
                                    pj[:, 0:128], lx, wqkv_sb[:, dt, g, 0:128],
                                    start=(dt == 0), stop=False)
                                nc.tensor.matmul(
                                    pj[:, 128:256], lx,
                                    wqkv_sb[:, dt, g, 128:256],
                                    start=False, stop=False)
                                nc.tensor.matmul(
                                    pj[:, 256:384], lx,
                                    wqkv_sb[:, dt, g, 256:384],
                                    start=False, stop=(dt == DT_N - 1))
                            # v blend -> vp
                            nc.vector.tensor_tensor(
                                vp[:, tt, 2 * g : 2 * g + 2, 0:DH],
                                pj[:, 256:384].rearrange(
                                    "p (h e) -> p h e", h=2),
                                ve_sb[:, tt, gc].rearrange(
                                    "p (h e) -> p h e", h=2),
                                mybir.AluOpType.add,
                            )
                            # raw q,k copy (normalized in place per half);
                            # explicitly DVE — ACT must stay free for exps
                            nc.vector.tensor_copy(
                                out=QKg[:, tt, :], in_=pj[:, 0:256])
                            nc.scalar.square(sqg[:, tt, :], pj[:, 0:256])
                            if tt % 4 != 3:
                                continue
                            # ------ per-half RMS scale + fused rotary -------
                            hf = tt // 4
                            th = ds(4 * hf, 4)
                            ms = statp.tile([P, 4, 4], f16, tag="ms")
                            with nc.allow_low_precision(
                                    reason="sumsq of 64 fp16 values"):
                                nc.vector.reduce_sum(
                                    ms[:],
                                    sqg[:, th].rearrange(
                                        "p t (h e) -> p t h e", h=4),
                                    axis=mybir.AxisListType.X,
                                )
                            lnv = statp.tile([P, 4, 4], f32, tag="lnv")
                            nc.scalar.activation(
                                lnv[:], ms[:],
                                mybir.ActivationFunctionType.Ln,
                                bias=eps_sb[:], scale=1.0 / DH,
                            )
                            scl = statp.tile([P, 4, 4], f16, tag="scl")
                            nc.scalar.activation(
                                scl[:], lnv[:],
                                mybir.ActivationFunctionType.Exp,
                                scale=-0.5,
                            )
                            nc.vector.tensor_tensor(
                                QKg[:, th].rearrange(
                                    "p t (h e) -> p t h e", h=4),
                                QKg[:, th].rearrange(
                                    "p t (h e) -> p t h e", h=4),
                                scl[:, :, :, None].to_broadcast((P, 4, 4, DH)),
                                mybir.AluOpType.mult,
                            )
                            # fused q+k rotary: 8 (tensor,head,half) groups of
                            # 16 rotating lanes, one DVE op set
                            rot = QKg[:, th].rearrange(
                                "p t (hh eh e) -> p t hh eh e", hh=8, eh=2,
                            )[:, :, :, 0, :]  # [P, 4, 8, 16]
                            qsw = statp.tile([P, 4, 8, 16], f16, tag="qsw")
                            nc.vector.tensor_copy(
                                qsw[:, :, 0::2, :], rot[:, :, 1::2, :])
                            nc.vector.tensor_copy(
                                qsw[:, :, 1::2, :], rot[:, :, 0::2, :])
                            cosv = cos_sb[:, th].rearrange(
                                "p t (hh e) -> p t hh e", hh=8)
                            sinv = sin_sb[:, th].rearrange(
                                "p t (hh e) -> p t hh e", hh=8)
                            t1 = statp.tile([P, 4, 8, 16], f16, tag="t1")
                            nc.vector.tensor_tensor(
                                t1[:], rot, cosv, mybir.AluOpType.mult)
                            t2 = statp.tile([P, 4, 8, 16], f16, tag="t2")
                            nc.vector.tensor_tensor(
                                t2[:], qsw[:], sinv, mybir.AluOpType.mult)
                            nc.vector.tensor_tensor(
                                rot, t1[:], t2[:], mybir.AluOpType.add)
                            # ------ write half + xbar-transpose q,k ---------
                            nc.sync.dma_start(
                                out=qk_dr_v[:, th, :], in_=QKg[:, th, :])
                            hs = ds(512 * hf, 512)
                            nc.sync.dma_start_transpose(
                                QT_sb[:, g, hs], qk_dr[hs, 0:P])
                            nc.sync.dma_start_transpose(
                                KT_sb[:, g, hs], qk_dr[hs, P : 2 * P])

                        # ---------- scores^T + exp (causal ramp mask) -------
                        ET = etp.tile([P, 2, etoff[KT_N]], f16, tag="et")
                        et_tiles[g] = ET
                        for kt in range(KT_N):
                            qlo = kt * P
                            for qh in range(2):
                                qs = max(qh * 512, qlo)
                                qe = (qh + 1) * 512
                                if qs >= qe:
                                    continue
                                # per-512-chunk psum (2 banks) so scores(kt+1)
                                # overlaps exp(kt) with bufs=2
                                pst = psST.tile([P, 2, 512], f32, tag="st")
                                diag = qs == qlo
                                for hb in range(2):
                                    lo, hi = hb * 64, hb * 64 + 64
                                    nc.tensor.matmul(
                                        pst[:, hb, 0 : qe - qs],
                                        KT_sb[lo:hi, g, ts(kt, P)],
                                        QT_sb[lo:hi, g, ds(qs, qe - qs)],
                                        start=True, stop=not diag,
                                    )
                                    if diag:
                                        # additive -C*max(0, k-q) ramp kills
                                        # the upper triangle under exp
                                        nc.tensor.matmul(
                                            pst[:, hb, 0:P],
                                            am_sb[:], bm_sb[:],
                                            start=False, stop=True,
                                        )
                                nc.scalar.activation(
                                    ET[:, :, ds(etoff[kt] + qs - qlo, qe - qs)],
                                    pst[:, :, 0 : qe - qs],
                                    mybir.ActivationFunctionType.Exp,
                                    scale=ATTN_SCALE,
                                )
                        # ------- delayed AV: previous pair (sw pipeline) ----
                        if g > 0:
                            emit_av(g - 1)
                    emit_av(NG - 1)
                # ================= output projection =====================
                # per-(tt,ic) staging + stores alternating Pool/SP queues;
                # deep psF so jt=0..2 accumulation hoists under the pair-3 tail
                with tc.tile_pool(name="outps", bufs=6, space="PSUM") as psF, \
                     tc.tile_pool(name="outstage", bufs=4) as osp:
                    for tt in range(TT_N):
                        for ic in range(2):
                            po = psF.tile([P, 512], f32, tag="po")
                            for jt in range(JT_N):
                                nc.tensor.matmul(
                                    po[:],
                                    yT_sb[:, jt, ts(tt, P)],
                                    wo_sb[:, jt, ds(ic * 512, 512)],
                                    start=(jt == 0), stop=(jt == JT_N - 1),
                                )
                            osb = osp.tile([P, 512], f32, tag="osb")
                            nc.scalar.copy(out=osb[:], in_=po[:])
                            eng = nc.gpsimd if ic == 0 else nc.sync
                            eng.dma_start(
                                out=out_v[:, tt, ds(ic * 512, 512)],
                                in_=osb[:])

    split_sync_waits(nc)
    return nc


def make_core_inputs(x, qkvo_w, value_embeds, lambda_v):
    """Host-side prep: returns list of per-core input dicts (fp16)."""
    x = np.asarray(x)
    qkvo_w = np.asarray(qkvo_w)
    value_embeds = np.asarray(value_embeds)
    lambda_v = np.asarray(lambda_v)

    freq = (1.0 / 1024.0) ** np.linspace(0.0, 1.0, DH // 4, dtype=np.float32)
    theta = np.arange(T, dtype=np.float32)[:, None] * freq[None, :]  # [T, 16]
    cos = np.cos(theta).astype(np.float32)
    sin = np.sin(theta).astype(np.float32)
    # [T, 128] tiled over all 8 (q|k, head, half) groups: cos repeats, sin
    # alternates sign; then re-tiled to [P, TT_N*128] (partition-contiguous)
    cos128 = np.concatenate([cos] * 8, axis=1).astype(np.float16)
    sin128 = np.concatenate([sin, -sin] * 4, axis=1).astype(np.float16)
    cosP = cos128.reshape(TT_N, P, P).transpose(1, 0, 2).reshape(P, TT_N * P)
    sinP = sin128.reshape(TT_N, P, P).transpose(1, 0, 2).reshape(P, TT_N * P)
    cosP = np.ascontiguousarray(cosP)
    sinP = np.ascontiguousarray(sinP)
    # additive causal ramp mask: (amask.T @ bmask)[k, q] = -2000*max(0, k-q)
    jj = np.arange(P)
    amask_np = (jj[None, :] >= jj[:, None]).astype(np.float16)   # [j, k]
    bmask_np = (-2000.0 * (jj[:, None] > jj[None, :])).astype(np.float16)

    in_maps = []
    for c in range(N_CORES):
        b, hh = c // 2, c % 2
        R = slice(hh * H8 * DH, (hh + 1) * H8 * DH)
        wq = qkvo_w[0][R].T  # [D, 512]
        wk = qkvo_w[1][R].T
        wv = (lambda_v[0] * qkvo_w[2][R]).T
        # [D, NG, 384]: per pair the 128 q cols, 128 k cols, 128 v cols
        wqkv = np.empty((D, NG, 384), dtype=np.float16)
        for g in range(NG):
            wqkv[:, g, 0:128] = wq[:, g * 128 : (g + 1) * 128]
            wqkv[:, g, 128:256] = wk[:, g * 128 : (g + 1) * 128]
            wqkv[:, g, 256:384] = wv[:, g * 128 : (g + 1) * 128]
        # block-transpose: xTt[tt*128+p, dt*128+j] = x[b][tt*128+j, dt*128+p]
        xb = x[b].reshape(TT_N, P, DT_N, P)
        xTt = np.ascontiguousarray(
            xb.transpose(0, 3, 2, 1).reshape(T, D)).astype(np.float16)
        in_maps.append({
            "xTt": xTt,
            "wqkv": wqkv,
            "woT": np.ascontiguousarray(qkvo_w[3][:, R].T).astype(np.float16),
            "ve": (lambda_v[1] * value_embeds[:T, R]).astype(np.float16),
            "cosd": cosP,
            "sind": sinP,
            "amask": amask_np,
            "bmask": bmask_np,
        })
    return in_maps


_NC_CACHE = {}


def _get_nc(reps=1):
    if reps not in _NC_CACHE:
        _NC_CACHE[reps] = build_nc(reps)
    return _NC_CACHE[reps]


def kernel(x, qkvo_w, value_embeds, lambda_v):
    from concourse.bass_utils import run_bass_kernel_spmd

    nc = _get_nc()
    in_maps = make_core_inputs(x, qkvo_w, value_embeds, lambda_v)
    res = run_bass_kernel_spmd(nc, in_maps, list(range(N_CORES))).results
    out = np.empty((B, T, D), dtype=np.float32)
    for b in range(B):
        out[b] = res[2 * b]["out"] + res[2 * b + 1]["out"]
    return out
